# revision 2
# baseline (speedup 1.0000x reference)
"""Deep Richardson-Lucy deconvolution on 8 Trainium2 NeuronCores.

Strategy (per core, data-parallel batch shard of 512 rows):
- Everything lives in SBUF in a TRANSPOSED fp16 layout: [l on partitions
  (64 blocks of 128), batch on the free dim (512)].
- conv1d(K=31, zero-pad) == banded-Toeplitz matmul per 128-l block:
  one center [128,128] matmul + two 32-row halo matmuls against the
  neighbor blocks, packed to array corners via tile_position.
- Per RL iteration: conv(s) -> PSUM; r = ACT.Reciprocal(psum + EPS);
  ratio = m * r (DVE fp16 2x); conv(ratio, flipped) -> PSUM;
  s *= psum (DVE, PSUM operand).
- In/out transposes ride the DMA xbar transpose engine (fp16).
"""
import hashlib
import numpy as np

EPS = 1e-6
P = 128
KTAPS = 31
PAD = 15
B_FULL, L = 4096, 8192
N_CORES = 8
BC = B_FULL // N_CORES          # 512 batch rows per core
NT = L // P                     # 64 l-blocks
NITER = 10

_cache = {}


def _build_toeplitz(psf):
    Wc = np.zeros((P, P), dtype=np.float64)
    j = np.arange(P)[:, None]
    i = np.arange(P)[None, :]
    k = j - i + PAD
    m = (k >= 0) & (k < KTAPS)
    Wc[m] = psf[k[m]]
    WL = np.zeros((32, 32), dtype=np.float64)   # rhs = prev block parts [96,128)
    jj = np.arange(32)[:, None]
    ii = np.arange(32)[None, :]
    k = (96 + jj - 128) - ii + PAD
    m = (k >= 0) & (k < KTAPS)
    WL[m] = psf[k[m]]
    WR = np.zeros((32, 32), dtype=np.float64)   # rhs = next block parts [0,32)
    k = (jj + 128) - (96 + ii) + PAD
    m = (k >= 0) & (k < KTAPS)
    WR[m] = psf[k[m]]
    return Wc, WL, WR


def _wpack(psf):
    Wc, WL, WR = _build_toeplitz(psf)
    w = np.zeros((P, 192), dtype=np.float16)
    w[:, 0:128] = Wc
    w[96:128, 128:160] = WL
    w[0:32, 160:192] = WR
    return w


def _r0pack(psf64):
    """r0[p, t] = 1 / (conv1d(0.5*ones, psf)[128t+p] + EPS)."""
    ones = np.full((1, L), 0.5, dtype=np.float64)
    xp = np.pad(ones, ((0, 0), (PAD, PAD)))
    sc = np.zeros((1, L), dtype=np.float64)
    for k in range(KTAPS):
        sc += xp[:, k:k + L] * psf64[k]
    r = 1.0 / (sc[0] + EPS)
    return r.reshape(NT, P).T.astype(np.float32)


def _build(psf64, alpha64):
    import concourse.bass as bass
    import concourse.tile as tile
    from concourse import mybir
    import bass_rust

    F32 = mybir.dt.float32
    F16 = mybir.dt.float16

    class SafeTC(tile.TileContext):
        # this walrus build rejects >1 sync wait per CTRL-class instruction
        def _drain_and_barrier(self, tick_clock, wait_clock):
            gc = tick_clock.global_clock
            for i in range(len(gc)):
                if gc[i] > 0:
                    di = self.nc.sync.drain()
                    pc = bass_rust.VectorClock()
                    pc.require_at_least(i, gc[i])
                    wait_clock.add_sem_waits(di.ins, bass_rust.ScopedClock({None: pc}))
            self.nc.all_engine_barrier()
            popped = self.nc._tile_sem_poison_stack.pop()
            assert popped is self._sem_poison
            self.nc.clear_and_free_semaphores(list(self.sems.allocated().values()))
            self.nc.all_engine_barrier()

    def split_multi_waits(nc, max_waits=1):
        n_fixed = 0
        uid = [0]
        for f in nc.m.functions:
            for bb in f.blocks:
                out = []
                changed = False
                for inst in bb.instructions:
                    si = inst.sync_info
                    if si is not None:
                        sems = [w for w in si.on_wait
                                if str(getattr(w, "sync_type", "")) == "semaphore"]
                        other = [w for w in si.on_wait if w not in sems]
                        if len(sems) > max_waits:
                            keep = sems[-max_waits:]
                            for w in sems[:-max_waits]:
                                nop = mybir.InstNoOp(
                                    name=f"waitsplit_{uid[0]}", ins=[], outs=[])
                                uid[0] += 1
                                nop.engine = inst.engine
                                nop.sync_info = mybir.SyncInfo(
                                    on_wait=[w], on_update=[])
                                out.append(nop)
                            inst.sync_info = mybir.SyncInfo(
                                on_wait=other + keep,
                                on_update=list(si.on_update))
                            n_fixed += 1
                            changed = True
                    out.append(inst)
                if changed:
                    try:
                        bb.instructions = out
                    except Exception:
                        bb.instructions.clear()
                        bb.instructions.extend(out)
        return n_fixed

    def act_raw(nc, out, in_, func, bias=0.0, scale=1.0):
        eng = nc.scalar
        ins = [eng.lower_ap(in_),
               mybir.ImmediateValue(dtype=F32, value=float(bias)),
               mybir.ImmediateValue(dtype=F32, value=float(scale)),
               mybir.ImmediateValue(dtype=F32, value=0.0)]
        return eng.add_instruction(mybir.InstActivation(
            name=nc.get_next_instruction_name(), func=func, ins=ins,
            outs=[eng.lower_ap(out)]))

    alpha_is_one = bool(np.all(alpha64 == 1.0))

    nc = bass.Bass("TRN2", target_bir_lowering=False, debug=False,
                   num_devices=N_CORES)
    m_in = nc.dram_tensor("m", [BC, L], F32, kind="ExternalInput")
    w1_in = nc.dram_tensor("w1", [P, 192], F16, kind="ExternalInput")
    w2_in = nc.dram_tensor("w2", [P, 192], F16, kind="ExternalInput")
    r0_in = nc.dram_tensor("r0", [P, NT], F32, kind="ExternalInput")
    y_out = nc.dram_tensor("y", [BC, L], F32, kind="ExternalOutput")

    Rec = mybir.ActivationFunctionType.Reciprocal
    Ln = mybir.ActivationFunctionType.Ln
    Exp = mybir.ActivationFunctionType.Exp

    def conv_block(psum, w, src, t, start_grp):
        last = "R" if t < NT - 1 else ("L" if t > 0 else "C")
        nc.tensor.matmul(psum[:], w[:, 0:128], src[:, t, :],
                         start=start_grp, stop=(last == "C"))
        if t > 0:
            nc.tensor.matmul(psum[0:32, :], w[96:128, 128:160],
                             src[96:128, t - 1, :], start=False,
                             stop=(last == "L"), tile_position=(96, 0))
        if t < NT - 1:
            nc.tensor.matmul(psum[96:128, :], w[0:32, 160:192],
                             src[0:32, t + 1, :], start=False,
                             stop=(last == "R"), tile_position=(0, 96))

    with SafeTC(nc) as tc:
        with tc.tile_pool(name="wpool", bufs=1) as wpool, \
             tc.tile_pool(name="mpool", bufs=1) as mpool, \
             tc.tile_pool(name="spool", bufs=1) as spool:
            w1 = wpool.tile([P, 192], F16)
            nc.sync.dma_start(w1[:], w1_in[:])
            w2 = wpool.tile([P, 192], F16)
            nc.sync.dma_start(w2[:], w2_in[:])
            r0 = wpool.tile([P, NT], F32)
            nc.sync.dma_start(r0[:], r0_in[:])
            mT = mpool.tile([P, NT, BC], F16)
            s = spool.tile([P, NT, BC], F16)
            nc.vector.memset(s[:], 0.5)

            # ---- load m, cast fp16, DMA-xbar transpose into mT ----
            with tc.tile_pool(name="stage", bufs=1) as stage:
                for c in range(BC // P):
                    st32 = stage.tile([P, L], F32, tag="st32")
                    nc.sync.dma_start(st32[:], m_in[c * P:(c + 1) * P, :])
                    st16 = stage.tile([P, L], F16, tag="st16")
                    nc.vector.tensor_copy(st16[:], st32[:])
                    nc.sync.dma_start_transpose(
                        mT[:, :, c * P:(c + 1) * P], st16[:])

            # ---- RL iterations ----
            with tc.tile_pool(name="ratio", bufs=8) as rpool, \
                 tc.tile_pool(name="rtile", bufs=4) as rtp, \
                 tc.tile_pool(name="psum", bufs=6, space="PSUM") as pp:
                for it in range(NITER):
                    ratio_tiles = [None] * NT

                    def _ratio(t):
                        ra = rpool.tile([P, BC], F16, tag="ra")
                        if it == 0:
                            # s == 0.5 everywhere: conv(s)+EPS is a per-l
                            # constant; r0 = 1/that, precomputed on host.
                            nc.vector.tensor_scalar(
                                out=ra[:], in0=mT[:, t, :],
                                scalar1=r0[:, t:t + 1], scalar2=None,
                                op0=mybir.AluOpType.mult)
                        else:
                            ps = pp.tile([P, BC], mybir.dt.float32, tag="ps")
                            conv_block(ps, w1, s, t, True)
                            rt = rtp.tile([P, BC], F16, tag="rt")
                            act_raw(nc, rt[:], ps[:], Rec, bias=EPS)
                            nc.vector.tensor_mul(ra[:], mT[:, t, :], rt[:])
                        ratio_tiles[t] = ra

                    def _conv2_update(t):
                        ps = pp.tile([P, BC], mybir.dt.float32, tag="ps")
                        last = "R" if t < NT - 1 else "L"
                        nc.tensor.matmul(ps[:], w2[:, 0:128],
                                         ratio_tiles[t][:], start=True,
                                         stop=False)
                        if t > 0:
                            nc.tensor.matmul(
                                ps[0:32, :], w2[96:128, 128:160],
                                ratio_tiles[t - 1][96:128, :], start=False,
                                stop=(last == "L"), tile_position=(96, 0))
                        if t < NT - 1:
                            nc.tensor.matmul(
                                ps[96:128, :], w2[0:32, 160:192],
                                ratio_tiles[t + 1][0:32, :], start=False,
                                stop=(last == "R"), tile_position=(0, 96))
                        if alpha_is_one:
                            if t % 2 == 0:
                                # DVE fused: s = (psum + EPS) * s, PSUM src 1x
                                nc.vector.scalar_tensor_tensor(
                                    out=s[:, t, :], in0=ps[:], scalar=EPS,
                                    in1=s[:, t, :],
                                    op0=mybir.AluOpType.add,
                                    op1=mybir.AluOpType.mult)
                            else:
                                # ACT evacuates PSUM (+EPS), DVE fp16 mul 2x
                                cp = rtp.tile([P, BC], F16, tag="cp")
                                act_raw(nc, cp[:], ps[:],
                                        mybir.ActivationFunctionType.Copy,
                                        bias=EPS)
                                nc.vector.tensor_mul(s[:, t, :], s[:, t, :],
                                                     cp[:])
                        else:
                            lg = rtp.tile([P, BC], F32, tag="lg")
                            act_raw(nc, lg[:], ps[:], Ln, bias=EPS)
                            cp = rtp.tile([P, BC], F16, tag="cp")
                            act_raw(nc, cp[:], lg[:], Exp,
                                    scale=float(alpha64[it]))
                            nc.vector.tensor_mul(s[:, t, :], s[:, t, :], cp[:])

                    # software-pipelined emission: keeps per-engine FIFO
                    # order producer/consumer-coupled so pool slots recycle
                    # without cross-engine ordering cycles.
                    _ratio(0)
                    _ratio(1)
                    for w in range(NT):
                        if w + 2 < NT:
                            _ratio(w + 2)
                        _conv2_update(w)

            # ---- transpose back + cast fp32 + store ----
            with tc.tile_pool(name="outp", bufs=1) as outp:
                for q in range(4):
                    sn16 = outp.tile([P, NT, P], F16, tag="sn16")
                    nc.sync.dma_start_transpose(sn16[:], s[:, q * 16:(q + 1) * 16, :])
                    sn32 = outp.tile([P, NT, P], F32, tag="sn32")
                    nc.vector.tensor_copy(sn32[:], sn16[:])
                    sn32r = sn32.rearrange("p (tl bc) lp -> p tl bc lp",
                                           tl=16, bc=4)
                    for bc in range(4):
                        ydst = y_out[bc * P:(bc + 1) * P,
                                     q * 2048:(q + 1) * 2048].rearrange(
                            "p (tl lp) -> p tl lp", lp=P)
                        nc.sync.dma_start(ydst, sn32r[:, :, bc, :])

    split_multi_waits(nc)
    return nc


def _make_in_maps(m, psf, alpha):
    m = np.asarray(m)
    psf64 = np.asarray(psf, dtype=np.float64)
    w1 = _wpack(psf64)
    w2 = _wpack(psf64[::-1])
    r0 = _r0pack(psf64)
    return [{"m": np.ascontiguousarray(m[c * BC:(c + 1) * BC]).astype(np.float32),
             "w1": w1, "w2": w2, "r0": r0} for c in range(N_CORES)]


def kernel(m, psf, alpha):
    m = np.asarray(m)
    psf64 = np.asarray(psf, dtype=np.float64)
    alpha64 = np.asarray(alpha, dtype=np.float64)
    key = hashlib.sha256(
        psf64.tobytes() + alpha64.tobytes() + str(m.shape).encode()).hexdigest()
    if key not in _cache:
        _cache[key] = _build(psf64, alpha64)
    nc = _cache[key]

    from concourse.bass_utils import run_bass_kernel_spmd
    in_maps = _make_in_maps(m, psf, alpha)
    res = run_bass_kernel_spmd(nc, in_maps, core_ids=list(range(N_CORES)))
    out = np.concatenate([res.results[c]["y"] for c in range(N_CORES)], axis=0)
    return out.astype(np.float32)

